# revision 1
# baseline (speedup 1.0000x reference)
"""Trainium2 Bass kernel for nn_DocumentGNN (2-layer GCN, N=100K, E=1.2M).

Strategy (8 NeuronCores, SPMD):
  - Nodes permuted (degree-balanced snake deal) and sharded NL/core.
  - Stage A (redundant on every core): h0d = dinv*relu(x_aug@We_aug) for ALL
    nodes via bf16 DMA-transpose of x + PE matmuls -> local HBM table (f32).
    (dinv prescaled into x_aug on host; bias via augmented ones column.)
  - Conv layer: edge slots grouped by (dst-tile, src-quarter); values fetched
    with dma_gather (int16 idx within a NTOT/4-row quarter, 4 SWDGE queues);
    per 128-slot chunk a one-hot lhsT is built on DVE (is_equal vs iota) and
    a PE matmul accumulates into the tile's PSUM. Self-loop rows via one
    regular DMA per tile-group + identity matmul.
  - Then per tile: S -> transpose (PE) -> z = S @ W' (BN scale folded into
    W'), fused dinv-scale + bias + relu epilogue (DVE/ACT).
  - Layer-1 output h1d (bf16) AllGathered; layer 2 repeats the conv; final
    classifier matmul per tile. Host un-permutes the output.
"""

import numpy as np
import ml_dtypes

import concourse.bass as bass
import concourse.mybir as mybir
import concourse.tile as tile
from concourse import bacc
from concourse.bass import ds
from concourse.bass_utils import run_bass_kernel_spmd

F32 = mybir.dt.float32
BF16 = mybir.dt.bfloat16
I16 = mybir.dt.int16

F_IN = 22
H = 64
C_OUT = 6
EPS = 1e-5
NCORES = 8
TG = 8          # tiles per gather call-group
GB = 4          # one-hot chunks per DVE build op
MAXI = 8192     # max idxs per dma_gather call

# problem size (overridable for small-scale sim testing)
N = 100000
E = 1200000


def _sizes(n):
    nused = -(-n // NCORES)
    nl = -(-nused // 128) * 128
    if nl == nused:
        nl += 128  # always keep pad rows (zero-row sentinel lives there)
    return nused, nl, nl // 128, NCORES * nl


NUSED, NL, NT, NTOT = _sizes(N)
QROWS = NTOT // 4

_cache = {}
_patched = False


def _patch_dma_gather():
    """Relax dma_gather's elem granularity from 256B to 128B (the 256B
    assert guards the transpose path; non-transpose descriptors handle
    128B fine)."""
    global _patched
    if _patched:
        return
    import inspect
    import textwrap
    import concourse.bass as cb
    src = inspect.getsource(cb.BassGpSimd.dma_gather)
    src = textwrap.dedent(src).replace(
        "elem_size_bytes > 0 and elem_size_bytes % 256 == 0",
        "elem_size_bytes > 0 and elem_size_bytes % 128 == 0")
    src = src.replace("def dma_gather(", "def _patched_dma_gather(")
    ns = dict(cb.__dict__)
    exec(compile(src, "<dma_gather_patch>", "exec"), ns)
    cb.BassGpSimd.dma_gather = ns["_patched_dma_gather"]
    _patched = True


def set_size(n, e):
    """Testing hook: shrink the problem (n divisible by 8)."""
    global N, E, NUSED, NL, NT, NTOT, QROWS
    N, E = n, e
    NUSED, NL, NT, NTOT = _sizes(n)
    QROWS = NTOT // 4
    assert QROWS <= 32768
    _cache.clear()


# ---------------------------------------------------------------- host prep

def _wrap_idx(flat):
    n = len(flat)
    a = np.zeros((16, n // 16), np.int16)
    a[np.arange(n) % 16, np.arange(n) // 16] = flat
    return np.tile(a, (8, 1))


def host_prep(x, edge_index, W_emb, b_emb, W1, b1, g1, be1, m1, v1,
              W2, b2, g2, be2, m2, v2, W_cls, b_cls):
    x = np.asarray(x)
    src = np.asarray(edge_index[0]).astype(np.int64)
    dst = np.asarray(edge_index[1]).astype(np.int64)
    deg = np.bincount(dst, minlength=N).astype(np.int64)
    dinv = (1.0 / np.sqrt(deg + 1.0)).astype(np.float32)

    order = np.argsort(-deg, kind="stable")
    blk = np.arange(N) // NCORES
    pos = np.arange(N) % NCORES
    core_of_rank = np.where(blk % 2 == 0, pos, NCORES - 1 - pos)
    row = np.empty(N, np.int64)
    row[order] = core_of_rank * NL + blk
    assert blk.max() < NUSED

    r_src = row[src]
    r_dst = row[dst]
    core_e = r_dst // NL
    tile_e = (r_dst % NL) // 128
    dstoff_e = (r_dst % NL) % 128
    q_e = r_src // QROWS
    srcoff_e = r_src % QROWS
    zoff_q = np.array([NUSED + (q * QROWS) % NL for q in range(4)])
    # zero row inside each quarter: first core-block's pad row
    # quarter q starts at row q*QROWS; since QROWS = 2*NL, pad row of the
    # first core block inside quarter q is at offset NUSED.
    assert QROWS % NL == 0

    cnts = np.zeros((NCORES, NT, 4), np.int64)
    np.add.at(cnts, (core_e, tile_e, q_e), 1)
    ncnk = np.ceil(cnts / 128.0).astype(np.int64).max(axis=0)  # [NT, 4]

    groups = []
    t0 = 0
    while t0 < NT:
        t1 = min(t0 + TG, NT)
        while t1 > t0 + 1 and int((128 * ncnk[t0:t1].sum(axis=0)).max()) > MAXI:
            t1 -= 1
        groups.append((t0, t1))
        t0 = t1

    chunk_meta = []
    chunks_of_tile = [[] for _ in range(NT)]   # V-positions (gchunk)
    qof_chunk = []                             # quarter of each gchunk
    gchunk = 0
    for gi, (ta, tb) in enumerate(groups):
        qcalls = []
        base = gchunk
        for q in range(4):
            nch = int(ncnk[ta:tb, q].sum())
            start = gchunk
            for t in range(ta, tb):
                for _ in range(int(ncnk[t, q])):
                    chunks_of_tile[t].append(gchunk)
                    qof_chunk.append(q)
                    gchunk += 1
            qcalls.append((q, start, nch))
        chunk_meta.append({"tiles": (ta, tb), "base": base, "qcalls": qcalls})
    nchunk = gchunk
    nchunk_pad = max(GB, -(-nchunk // GB) * GB)
    # matmul-order position of each chunk (per-tile contiguous)
    mmcol = np.zeros(max(1, nchunk), np.int64)
    mp = 0
    for t in range(NT):
        for gc in chunks_of_tile[t]:
            mmcol[gc] = mp
            mp += 1
    assert mp == nchunk

    gidx = np.zeros((NCORES, 128, max(1, nchunk * 8)), np.int16)
    dstoff = np.zeros((NCORES, 128, nchunk_pad), ml_dtypes.bfloat16)

    ek = core_e * (NT * 4) + tile_e * 4 + q_e
    eorder = np.argsort(ek, kind="stable")
    counts_flat = np.bincount(ek, minlength=NCORES * NT * 4)
    starts = np.zeros_like(counts_flat)
    starts[1:] = np.cumsum(counts_flat)[:-1]

    for c in range(NCORES):
        for gi, (ta, tb) in enumerate(groups):
            for q, chstart, nch in chunk_meta[gi]["qcalls"]:
                if nch == 0:
                    continue
                flat = np.full(nch * 128, NUSED, np.int64)  # zero row
                doff = np.zeros(nch * 128, np.float32)
                p = 0
                for t in range(ta, tb):
                    k = c * (NT * 4) + t * 4 + q
                    cnt = int(counts_flat[k])
                    sl = eorder[starts[k]:starts[k] + cnt]
                    ncap = int(ncnk[t, q]) * 128
                    assert cnt <= ncap
                    flat[p:p + cnt] = srcoff_e[sl]
                    doff[p:p + cnt] = dstoff_e[sl]
                    p += ncap
                gidx[c, :, chstart * 8:(chstart + nch) * 8] = _wrap_idx(
                    flat.astype(np.int16))
                dm = doff.reshape(nch, 128)
                for ci in range(nch):
                    dstoff[c, :, mmcol[chstart + ci]] = dm[ci]

    # ---- constants / folded weights ----
    dinv_row = np.zeros(NTOT, np.float32)
    dinv_row[row] = dinv

    x_aug = np.zeros((NTOT, 32), np.float32)
    x_aug[row, :F_IN] = x
    x_aug[row, F_IN] = 1.0
    x_aug *= dinv_row[:, None]
    x_aug16 = np.ascontiguousarray(x_aug.T).astype(ml_dtypes.bfloat16)

    We_aug = np.zeros((32, H), np.float32)
    We_aug[:F_IN] = np.asarray(W_emb)
    We_aug[F_IN] = np.asarray(b_emb)

    def fold(W, b, g, be, m, v):
        a = (np.asarray(g) / np.sqrt(np.asarray(v) + EPS)).astype(np.float32)
        Wp = (np.asarray(W) * a[None, :]).astype(np.float32)
        Bp = ((np.asarray(b) - np.asarray(m)) * a + np.asarray(be)).astype(
            np.float32)
        return Wp, Bp

    W1p, B1p = fold(W1, b1, g1, be1, m1, v1)
    W2p, B2p = fold(W2, b2, g2, be2, m2, v2)

    dinvrep = np.zeros((NCORES, 128, NT * H), np.float32)
    for c in range(NCORES):
        dv = dinv_row[c * NL:(c + 1) * NL].reshape(NT, 128)
        dinvrep[c] = np.repeat(
            dv.T[:, :, None], H, axis=2).reshape(128, NT * H)

    B1rep = np.tile(B1p[None, :], (128, 8)).astype(np.float32)
    B2rep = np.tile(B2p[None, :], (128, 8)).astype(np.float32)
    Wcls_p = np.zeros((H, 8), np.float32)
    Wcls_p[:, :C_OUT] = np.asarray(W_cls)
    bcls_rep = np.zeros((128, 8), np.float32)
    bcls_rep[:, :C_OUT] = np.asarray(b_cls)[None, :]

    iota4 = np.tile(np.arange(128, dtype=np.float32)[None, :], (128, GB)).astype(ml_dtypes.bfloat16)
    ident32 = np.zeros((128, 128), np.float32)
    np.fill_diagonal(ident32, 1.0)

    meta = {
        "groups": groups,
        "chunk_meta": chunk_meta,
        "chunks_of_tile": chunks_of_tile,
        "nchunk": nchunk,
        "nchunk_pad": nchunk_pad,
        "mmcol": mmcol,
        "row": row,
    }
    shared = {
        "xT": x_aug16,
        "We": We_aug.astype(ml_dtypes.bfloat16),
        "W1p": W1p.astype(ml_dtypes.bfloat16),
        "W2p": W2p.astype(ml_dtypes.bfloat16),
        "Wcls": Wcls_p,
        "bclsrep": bcls_rep,
        "B1rep": B1rep,
        "B2rep": B2rep,
        "iota4": iota4,
        "ident32": ident32,
        "ident16": ident32.astype(ml_dtypes.bfloat16),
    }
    per_core = []
    for c in range(NCORES):
        d = dict(shared)
        d["dinvrep"] = dinvrep[c]
        d["gidx"] = gidx[c]
        d["dstoff"] = dstoff[c]
        per_core.append(d)
    return meta, per_core


# ---------------------------------------------------------------- program

def build_program(meta, flags=(), repeat=1):
    flags = set(flags)
    groups = meta["groups"]
    chunk_meta = meta["chunk_meta"]
    chunks_of_tile = meta["chunks_of_tile"]
    nchunk = meta["nchunk"]
    nchunk_pad = meta["nchunk_pad"]
    mmcol = meta["mmcol"]

    _patch_dma_gather()
    nc = bacc.Bacc(
        "TRN2", target_bir_lowering=False, debug=False,
        enable_asserts=False, num_devices=NCORES, num_swdge_queues=4,
    )

    xT_d = nc.dram_tensor("xT", [32, NTOT], BF16, kind="ExternalInput")
    We = nc.dram_tensor("We", [32, H], BF16, kind="ExternalInput")
    W1p = nc.dram_tensor("W1p", [H, H], BF16, kind="ExternalInput")
    W2p = nc.dram_tensor("W2p", [H, H], BF16, kind="ExternalInput")
    Wcls = nc.dram_tensor("Wcls", [H, 8], F32, kind="ExternalInput")
    bclsrep = nc.dram_tensor("bclsrep", [128, 8], F32, kind="ExternalInput")
    B1rep = nc.dram_tensor("B1rep", [128, 512], F32, kind="ExternalInput")
    B2rep = nc.dram_tensor("B2rep", [128, 512], F32, kind="ExternalInput")
    dinvrep_d = nc.dram_tensor(
        "dinvrep", [128, NT * H], F32, kind="ExternalInput")
    gidx_d = nc.dram_tensor(
        "gidx", [128, max(1, nchunk * 8)], I16, kind="ExternalInput")
    dstoff_d = nc.dram_tensor(
        "dstoff", [128, nchunk_pad], BF16, kind="ExternalInput")
    iota4_d = nc.dram_tensor("iota4", [128, GB * 128], BF16, kind="ExternalInput")
    ident32_d = nc.dram_tensor("ident32", [128, 128], F32, kind="ExternalInput")
    ident16_d = nc.dram_tensor("ident16", [128, 128], BF16, kind="ExternalInput")
    out_d = nc.dram_tensor("out", [NL, 8], F32, kind="ExternalOutput")

    h0d = nc.dram_tensor("h0d", [NTOT, 128], BF16, kind="Internal")
    h1d_shard = nc.dram_tensor("h1d_shard", [NL, 128], BF16, kind="Internal")
    h1d_full = nc.dram_tensor(
        "h1d_full", [NTOT, 128], BF16, kind="Internal", addr_space="Shared")

    with tile.TileContext(nc) as tc:
        with tc.tile_pool(name="const", bufs=1) as cpool:
            def cload(name, src, shape, dt):
                t = cpool.tile(shape, dt, name=name)
                nc.sync.dma_start(out=t[:], in_=src[:].opt())
                return t

            We_sb = cload("We_sb", We, [32, H], BF16)
            W1_sb = cload("W1_sb", W1p, [H, H], BF16)
            W2_sb = cload("W2_sb", W2p, [H, H], BF16)
            Wc_sb = cload("Wc_sb", Wcls, [H, 8], F32)
            bc_sb = cload("bc_sb", bclsrep, [128, 8], F32)
            B1_sb = cload("B1_sb", B1rep, [128, 512], F32)
            B2_sb = cload("B2_sb", B2rep, [128, 512], F32)
            dinv_sb = cload("dinv_sb", dinvrep_d, [128, NT * H], F32)
            gidx_sb = cload("gidx_sb", gidx_d, [128, max(1, nchunk * 8)], I16)
            doff_sb = cload("doff_sb", dstoff_d, [128, nchunk_pad], BF16)
            iota_sb = cload("iota_sb", iota4_d, [128, GB * 128], BF16)
            id32_sb = cload("id32_sb", ident32_d, [128, 128], F32)
            id16_sb = cload("id16_sb", ident16_d, [128, 128], BF16)

            pid = nc.sync.partition_id()
            selfoff = pid * NL

            def _iter():
                # ---------------- stage A ----------------
                with tc.tile_pool(name="stgA", bufs=2) as apool, \
                     tc.tile_pool(name="psA", bufs=2, space="PSUM") as pspool:
                    for cb in (() if "noA" in flags else range(NCORES)):
                        xT = apool.tile([32, NL], BF16, tag="xT", name="xT")
                        nc.sync.dma_start(
                            out=xT[:],
                            in_=xT_d[:, cb * NL:(cb + 1) * NL],
                        )
                        ngrp = (NT + 7) // 8
                        for g8 in range(ngrp):
                            ta, tb = g8 * 8, min(g8 * 8 + 8, NT)
                            w = (tb - ta) * H
                            ps = pspool.tile([128, 512], F32, tag="psA", name="psA")
                            for t in range(ta, tb):
                                j = t - ta
                                nc.tensor.matmul(
                                    out=ps[:, j * H:(j + 1) * H],
                                    lhsT=xT[:, t * 128:(t + 1) * 128],
                                    rhs=We_sb[:],
                                    start=True, stop=True,
                                )
                            stg = apool.tile([128, 512], BF16, tag="stgA",
                                             name="stgA", bufs=3)
                            nc.scalar.activation(
                                out=stg[:, :w], in_=ps[:, :w],
                                func=mybir.ActivationFunctionType.Relu,
                            )
                            rows0 = cb * NL + ta * 128
                            nc.sync.dma_start(
                                out=h0d[rows0:rows0 + (tb - ta) * 128, 0:H]
                                .rearrange("(k p) f -> p k f", p=128),
                                in_=stg[:, :w].rearrange("p (k f) -> p k f", f=H),
                            )

                # ---------------- conv layers ----------------
                def conv_layer(layer, vp, pp):
                    if layer == 1:
                        table, tdt, ident, W_sb, B_sb = h0d, BF16, id16_sb, W1_sb, B1_sb
                    else:
                        table, tdt, ident, W_sb, B_sb = (
                            h1d_full, BF16, id16_sb, W2_sb, B2_sb)

                    ohs = {}
                    for gi, (ta, tb) in enumerate(groups):
                        gm = chunk_meta[gi]
                        base = gm["base"]
                        ctg = sum(nch for _, _, nch in gm["qcalls"])
                        ntile = tb - ta
                        # gathers (4 queues in parallel)
                        vt = vp.tile([128, max(1, ctg), 128], tdt, tag="V",
                                     name="V", bufs=2)
                        for q, chstart, nch in gm["qcalls"]:
                            if nch == 0 or "nogather" in flags:
                                continue
                            ni = nch * 128
                            nc.gpsimd.dma_gather(
                                out_ap=vt[:, chstart - base:chstart - base + nch, :],
                                in_ap=table[q * QROWS:(q + 1) * QROWS, :],
                                idxs_ap=gidx_sb[:, chstart * 8:(chstart + nch) * 8],
                                num_idxs=ni, num_idxs_reg=ni, elem_size=128,
                                single_packet=False, queue_num=q,
                            )
                        # self rows
                        selfb = vp.tile([128, TG, 128], tdt, tag="selfb",
                                        name="selfb", bufs=2)
                        if layer == 1:
                            src_ap = h0d[
                                ds(selfoff + ta * 128, ntile * 128), :]
                        else:
                            src_ap = h1d_shard[ta * 128:ta * 128 + ntile * 128, :]
                        nc.sync.dma_start(
                            out=selfb[:, :ntile, :],
                            in_=src_ap.rearrange("(k p) f -> p k f", p=128),
                        )
                        # segsum + per-tile tail
                        if "nomm" in flags:
                            continue
                        zgrp = pp.tile([128, 512], F32, tag="z", name="z", bufs=2)
                        for t in range(ta, tb):
                            j = t - ta
                            sps = pp.tile([128, H], F32, tag="S", name="S", bufs=2)
                            chl = chunks_of_tile[t]
                            nc.tensor.matmul(
                                out=sps[:], lhsT=ident[:], rhs=selfb[:, j, 0:H],
                                start=True, stop=(len(chl) == 0),
                            )
                            for k, gc in enumerate(chl if "noedges" not in flags else []):
                                mc = int(mmcol[gc])
                                b0 = mc - mc % GB
                                if b0 not in ohs:
                                    oh = vp.tile([128, GB, 128], tdt, tag="oh",
                                                 name="oh", bufs=3)
                                    nc.vector.tensor_tensor(
                                        out=oh[:],
                                        in0=iota_sb[:].rearrange(
                                            "p (g c) -> p g c", g=GB),
                                        in1=doff_sb[:, b0:b0 + GB]
                                        .rearrange("p (g o) -> p g o", o=1)
                                        .to_broadcast([128, GB, 128]),
                                        op=mybir.AluOpType.is_equal,
                                    )
                                    ohs[b0] = oh
                                nc.tensor.matmul(
                                    out=sps[:],
                                    lhsT=ohs[b0][:, mc % GB, :],
                                    rhs=vt[:, gc - base, 0:H],
                                    start=False, stop=(k == len(chl) - 1),
                                )
                            ssb = vp.tile([128, H], BF16, tag="Ssb", name="Ssb",
                                          bufs=4)
                            nc.scalar.copy(out=ssb[:], in_=sps[:])
                            sT = pp.tile([64, 128], BF16, tag="sT", name="sT",
                                         bufs=1)
                            nc.tensor.transpose(
                                out=sT[:], in_=ssb[:], identity=id16_sb[:])
                            sTs = vp.tile([64, 128], BF16, tag="sTs", name="sTs",
                                          bufs=4)
                            nc.vector.tensor_copy(out=sTs[:], in_=sT[:])
                            nc.tensor.matmul(
                                out=zgrp[:, j * H:(j + 1) * H],
                                lhsT=sTs[:], rhs=W_sb[:],
                                start=True, stop=True,
                            )
                        # fused epilogue for this group
                        w = ntile * H
                        g0 = ta * H
                        t1 = vp.tile([128, 512], F32, tag="post1", name="post1",
                                     bufs=3)
                        nc.vector.tensor_tensor(
                            out=t1[:, :w], in0=zgrp[:, :w],
                            in1=dinv_sb[:, g0:g0 + w], op=mybir.AluOpType.mult)
                        nc.vector.tensor_tensor(
                            out=t1[:, :w], in0=t1[:, :w], in1=B_sb[:, :w],
                            op=mybir.AluOpType.add)
                        h = vp.tile([128, 512], F32, tag="post2", name="post2",
                                    bufs=3)
                        nc.scalar.activation(
                            out=h[:, :w], in_=t1[:, :w],
                            func=mybir.ActivationFunctionType.Relu)
                        if layer == 1:
                            hd = vp.tile([128, 512], BF16, tag="post3",
                                         name="post3", bufs=3)
                            nc.vector.tensor_tensor(
                                out=hd[:, :w], in0=h[:, :w],
                                in1=dinv_sb[:, g0:g0 + w],
                                op=mybir.AluOpType.mult)
                            nc.sync.dma_start(
                                out=h1d_shard[ta * 128:ta * 128 + ntile * 128, 0:H]
                                .rearrange("(k p) f -> p k f", p=128),
                                in_=hd[:, :w].rearrange("p (k f) -> p k f", f=H),
                            )
                        else:
                            for j in range(ntile):
                                t = ta + j
                                hT = pp.tile([64, 128], F32, tag="hT", name="hT",
                                             bufs=1)
                                nc.tensor.transpose(
                                    out=hT[:], in_=h[:, j * H:(j + 1) * H],
                                    identity=id32_sb[:])
                                hTs = vp.tile([64, 128], F32, tag="hTs",
                                              name="hTs", bufs=2)
                                nc.vector.tensor_copy(out=hTs[:], in_=hT[:])
                                lg = pp.tile([128, 8], F32, tag="lg", name="lg",
                                             bufs=1)
                                nc.tensor.matmul(
                                    out=lg[:], lhsT=hTs[:], rhs=Wc_sb[:],
                                    start=True, stop=True)
                                lo = vp.tile([128, 8], F32, tag="lo", name="lo",
                                             bufs=3)
                                nc.vector.tensor_tensor(
                                    out=lo[:], in0=lg[:], in1=bc_sb[:],
                                    op=mybir.AluOpType.add)
                                nc.sync.dma_start(
                                    out=out_d[t * 128:(t + 1) * 128, :].opt(),
                                    in_=lo[:])

                if "nocv1" not in flags:
                    with tc.tile_pool(name="cv1", bufs=1) as vp1, \
                         tc.tile_pool(name="cp1", bufs=1, space="PSUM") as pp1:
                        conv_layer(1, vp1, pp1)
                if "noag" not in flags:
                    nc.gpsimd.collective_compute(
                        "AllGather", mybir.AluOpType.bypass,
                        replica_groups=[list(range(NCORES))],
                        ins=[h1d_shard[:].opt()],
                        outs=[h1d_full[:].opt()],
                    )
                if "nocv2" not in flags:
                    with tc.tile_pool(name="cv2", bufs=1) as vp2, \
                         tc.tile_pool(name="cp2", bufs=1, space="PSUM") as pp2:
                        conv_layer(2, vp2, pp2)

            for _rep in range(repeat):
                _iter()

    nc.compile()
    return nc



# ---------------------------------------------------------------- kernel

def kernel(**inputs):
    meta, per_core = host_prep(**inputs)
    if "prog" not in _cache:
        _cache["prog"] = build_program(meta)
    nc = _cache["prog"]
    in_maps = [{k: np.asarray(v) for k, v in pc.items()} for pc in per_core]
    res = run_bass_kernel_spmd(nc, in_maps, core_ids=list(range(NCORES)))
    row = meta["row"]
    full = np.empty((NTOT, 8), np.float32)
    for c in range(NCORES):
        full[c * NL:(c + 1) * NL] = res.results[c]["out"]
    return np.ascontiguousarray(full[row, :C_OUT])


# ------------------------------------------------- numpy reference of algebra

def numpy_model(inputs):
    """f32 numpy replica of the device algebra (for factorization checks)."""
    meta, per_core = host_prep(**inputs)
    row = meta["row"]
    x_aug = per_core[0]["xT"].astype(np.float32).T
    We = per_core[0]["We"].astype(np.float32)
    h0d = np.maximum(x_aug @ We, 0.0)
    dinvrow = np.zeros((NTOT, 1), np.float32)
    for c in range(NCORES):
        dr = per_core[c]["dinvrep"][:, 0::H]  # [128, NT]
        dinvrow[c * NL:(c + 1) * NL, 0] = dr.T.reshape(-1)

    src = np.asarray(inputs["edge_index"][0]).astype(np.int64)
    dst = np.asarray(inputs["edge_index"][1]).astype(np.int64)
    rs, rd = row[src], row[dst]

    def conv(table, Wp, Brep):
        S = np.zeros((NTOT, H), np.float32)
        np.add.at(S, rd, table[rs])
        S += table
        z = S @ Wp
        y = z * dinvrow + Brep
        return np.maximum(y, 0.0)

    W1p = per_core[0]["W1p"].astype(np.float32)
    B1 = per_core[0]["B1rep"][0, :H].astype(np.float32)
    h1 = conv(h0d, W1p, B1[None, :])
    h1d = h1 * dinvrow
    W2p = per_core[0]["W2p"].astype(np.float32)
    B2 = per_core[0]["B2rep"][0, :H].astype(np.float32)
    h2 = conv(h1d, W2p, B2[None, :])
    Wc = per_core[0]["Wcls"].astype(np.float32)
    bc = per_core[0]["bclsrep"][0].astype(np.float32)
    logits = h2 @ Wc + bc[None, :]
    return np.ascontiguousarray(logits[row, :C_OUT])

